# revision 19
# baseline (speedup 1.0000x reference)
"""Trainium2 Bass kernel for GatedActivation (gate-conv3d + sigmoid gating).

Reference computation:
  x: [2, 120, 48, 48, 48] f32   (channels = 32 scalar + 16*3 (l=1) + 8*5 (l=2))
  w_gate: [24, 120, 5, 5, 5] f32
  g = sigmoid(conv3d(x, w_gate, same padding))         # [2, 24, 48, 48, 48]
  out[:, 0:32]    = relu(x[:, 0:32])
  out[:, 32+3m+d] = x[:, 32+3m+d] * g[:, m]            (m in 0..15)
  out[:, 80+5m+d] = x[:, 80+5m+d] * g[:, 16+m]         (m in 0..7)

Sharding: 8 cores = batch(2) x X-split(4 slabs of 12 planes).

Algorithm per core:
  - conv taps (a, b) = (kx, ky) with kz folded into the 120 stationary
    columns (kz*24+o).  The padded (y, z) plane (52*52=2704 flat cols) is
    streamed FLAT; the useful output region is flat cols [104, 2600),
    covered by 5 PSUM-bank chunks of 512/448 columns.  A tap's moving
    operand is the flat plane shifted by (b-2)*52 + chunk offset.
  - mixed precision: N_PAIRS tap-pairs (b0, b0+1) run as fp8e4m3 DoubleRow
    matmuls (moving [128, 2, N] carries both taps, 2 MACs/cell/cycle); the
    remaining taps run as fp16 matmuls.  Both accumulate into the same
    fp32 PSUM chunk banks.  Weights are pre-scaled by 512 so fp8 weights
    escape the e4m3 subnormal range; the esum selector divides it back.
    The fp8 share keeps conv quantization error inside the 2e-2 gate
    (err ~ 0.023 * sqrt(n8/25)).
  - fixup per PLANE (cuts DMA instruction count ~6x vs per-y-tile):
    5 ACT copies move the chunk banks to contiguous fp16 ss[128, 2496];
    5 gpsimd SBUF->SBUF DMAs extract per-kz-group z-aligned ss2[128,48,48]
    (group kz at partitions 24kz..24kz+24, source col 52*yy + z + kz);
    one esum matmul per y-tile sums the kz partials across partitions and
    broadcasts gates onto their 3/5 channels; sigmoid (ACT), gating
    multiply (DVE, fp16 out), relu overwrite (ACT), one output DMA per
    plane.  Output fp16, upcast to f32 on host.
"""

import sys

if "/opt/trn_rl_repo" not in sys.path:
    sys.path.insert(0, "/opt/trn_rl_repo")

import numpy as np

B = 2
C = 120
S = 48
K = 5
PAD = 2
NXS = 4          # x-axis shards
XS = S // NXS    # 12 output x-planes per core
XIN = XS + 2 * PAD   # 16 input planes per core
SP = S + 2 * PAD     # 52 padded y/z extent
FLAT = SP * SP       # 2704
NCO = 24
NSC = 32
CIN = 128
N_CORES = 8

N_PAIRS = 8          # fp8 DoubleRow tap pairs (2 taps each); rest fp16
WS = 512.0           # weight prescale (e4m3 subnormal avoidance)

CH_N = [512, 512, 512, 512, 448]                # chunk widths
CH_OFF = [104, 616, 1128, 1640, 2152]           # absolute flat offsets

_CACHE = {}


def _tap_split():
    pairs = []
    for a in range(K):
        for b0 in (0, 2):
            pairs.append((a, b0))
    pairs = pairs[:N_PAIRS]
    in_pair = {(a, b0 + i) for a, b0 in pairs for i in (0, 1)}
    taps16 = [(a, b) for a in range(K) for b in range(K) if (a, b) not in in_pair]
    return pairs, taps16


def _build(reps=1):
    import contextlib

    import concourse.tile as tile
    from concourse import bacc, mybir

    f32 = mybir.dt.float32
    f16 = mybir.dt.float16
    f8 = mybir.dt.float8e4

    pairs, taps16 = _tap_split()
    n16 = len(taps16)

    nc = bacc.Bacc("TRN2", target_bir_lowering=False, debug=False,
                   num_devices=N_CORES)
    xs16_d = nc.dram_tensor("xs16", [CIN, XIN, FLAT], f16, kind="ExternalInput").ap()
    xs8_d = nc.dram_tensor("xs8", [CIN, XIN, FLAT], f8, kind="ExternalInput").ap()
    w16_d = nc.dram_tensor("w16", [CIN, n16, 128], f16, kind="ExternalInput").ap()
    w8_d = nc.dram_tensor("w8", [CIN, max(N_PAIRS, 1), 2, 128], f8,
                          kind="ExternalInput").ap()
    esum_d = nc.dram_tensor("esum", [CIN, 6, 128], f16, kind="ExternalInput").ap()
    y_d = nc.dram_tensor("y", [C, XS, S, S], f16, kind="ExternalOutput").ap()

    with tile.TileContext(nc) as tc:
        with tc.tile_pool(name="wpool", bufs=1) as wpool, \
             tc.tile_pool(name="p16", bufs=7) as p16_pool, \
             tc.tile_pool(name="p8", bufs=7) as p8_pool, \
             tc.tile_pool(name="convps", bufs=5, space="PSUM") as conv_pool, \
             tc.tile_pool(name="sspool", bufs=2) as ss_pool, \
             tc.tile_pool(name="ss2pool", bufs=2) as ss2_pool, \
             tc.tile_pool(name="gpreps", bufs=3, space="PSUM") as gpre_pool, \
             tc.tile_pool(name="gsig", bufs=3) as gsig_pool, \
             tc.tile_pool(name="outpl", bufs=2) as out_pool:

            # weights go on the ACT queue so the SP queue starts streaming
            # plane loads immediately at kernel head.
            w16_t = wpool.tile([CIN, n16, 128], f16)
            nc.scalar.dma_start(w16_t[:], w16_d[:])
            w8_t = wpool.tile([CIN, max(N_PAIRS, 1), 2, 128], f8)
            nc.scalar.dma_start(w8_t[:], w8_d[:])
            esum_t = wpool.tile([CIN, 6, 128], f16)
            nc.scalar.dma_start(esum_t[:], esum_d[:])

            p16s = {}
            p8s = {}
            banks_of = {}
            ss2_of = {}

            def load_plane(q):
                t16 = p16_pool.tile([CIN, FLAT], f16, tag="p16", name=f"p16_{q}")
                nc.sync.dma_start(t16[:], xs16_d[:, q])
                t8 = p8_pool.tile([CIN, 2, FLAT], f8, tag="p8", name=f"p8_{q}")
                nc.scalar.dma_start(t8[:, 0, :], xs8_d[:, q])
                # slot 1 = same plane shifted one y-row (the DoubleRow pair
                # partner tap b+1); the tail is never read.
                nc.scalar.dma_start(t8[:, 1, 0:FLAT - SP], xs8_d[:, q, SP:FLAT])
                p16s[q] = t16
                p8s[q] = t8

            def conv(p):
                banks = [conv_pool.tile([CIN, n], f32, tag="cps",
                                        name=f"cps{p}_{c}")
                         for c, n in enumerate(CH_N)]
                banks_of[p] = banks
                # chunk-outer so each bank completes early (drain overlaps the
                # rest of the plane); fp16/fp8 taps ALTERNATE so every
                # DoubleRow LDWEIGHTS (256 cols, no FWL) hides under the
                # previous fp16 matmul's longer stream.
                ops = []
                for i in range(max(n16, len(pairs))):
                    if i < n16:
                        ops.append(("16", i, taps16[i]))
                    if i < len(pairs):
                        ops.append(("8", i, pairs[i]))
                nops = len(ops)
                for c, n in enumerate(CH_N):
                    for idx, (kind, i, tap) in enumerate(ops):
                        first = idx == 0
                        last = idx == nops - 1
                        if kind == "16":
                            a, b = tap
                            off = CH_OFF[c] + (b - PAD) * SP
                            nc.tensor.matmul(
                                banks[c][:], w16_t[:, i],
                                p16s[p + a][:, off:off + n],
                                start=first, stop=last)
                        else:
                            a, b0 = tap
                            off = CH_OFF[c] + (b0 - PAD) * SP
                            nc.tensor.matmul(
                                banks[c][:], w8_t[:, i],
                                p8s[p + a][:, :, off:off + n],
                                start=first, stop=last,
                                perf_mode=mybir.MatmulPerfMode.DoubleRow,
                                skip_group_check=True)

            ss_of = {}

            def stage1(p, shifts=True):
                banks = banks_of.pop(p)
                # 4 spare cols so the kz-shifted view below stays in-bounds
                ss = ss_pool.tile([CIN, 2500], f16, tag="ss", name=f"ss{p}")
                for c, n in enumerate(CH_N):
                    o = CH_OFF[c] - CH_OFF[0]
                    nc.scalar.copy(ss[:, o:o + n], banks[c][:])
                if not shifts:
                    ss_of[p] = ss
                    return
                # group kz keeps 52-wide rows: ss2[g, yy, z<48] = conv partial
                # for out (yy, z); src is one CONTIGUOUS 2496-col run per
                # partition (SWDGE descriptor count stays tiny).
                ss2 = ss2_pool.tile([CIN, S * SP], f16, tag="ss2", name=f"ss2{p}")
                for c in range(K):
                    np0 = NCO * c
                    np1 = NCO * (c + 1) if c < K - 1 else CIN
                    nc.gpsimd.dma_start(ss2[np0:np1, :],
                                        ss[np0:np1, c:c + S * SP])
                ss2_of[p] = ss2

            def stage2(p, direct=False):
                opl = out_pool.tile([C, S, S], f16, tag="opl", name=f"opl{p}")
                x16v = p16s[p + PAD][:].rearrange("p (y z) -> p y z", y=SP)
                if direct:
                    ss = ss_of.pop(p)
                else:
                    ss2 = ss2_of.pop(p)
                    ss2v = ss2[:].rearrange("p (y z) -> p y z", z=SP)
                for k in range(S // 8):
                    gpre = gpre_pool.tile([CIN, 8 * S], f32, tag="gpre",
                                          name=f"gpre{p}_{k}")
                    if direct:
                        # last plane: skip the shift DMAs; 5 accumulating
                        # matmuls with per-kz selector stationaries read the
                        # shifted windows straight out of ss.
                        for c in range(K):
                            mv = ss[:, 8 * SP * k + c:8 * SP * k + c + 8 * SP]
                            mv = mv.rearrange("p (y z) -> p y z", z=SP)[:, :, 0:S]
                            nc.tensor.matmul(
                                gpre[:], esum_t[:, 1 + c], mv,
                                start=(c == 0), stop=(c == K - 1))
                    else:
                        nc.tensor.matmul(
                            gpre[:], esum_t[:, 0], ss2v[:, k * 8:(k + 1) * 8, 0:S],
                            start=True, stop=True)
                    gsig = gsig_pool.tile([C, 8, S], f16, tag="gsig",
                                          name=f"gsig{p}_{k}")
                    nc.scalar.activation(
                        gsig[:], gpre[0:C, :].rearrange("p (y z) -> p y z", y=8),
                        mybir.ActivationFunctionType.Sigmoid)
                    nc.vector.tensor_mul(
                        opl[:, k * 8:k * 8 + 8, :],
                        x16v[0:C, k * 8 + PAD:k * 8 + PAD + 8, PAD:PAD + S],
                        gsig[:])
                    nc.vector.tensor_scalar_max(
                        opl[0:NSC, k * 8:k * 8 + 8, :],
                        x16v[0:NSC, k * 8 + PAD:k * 8 + PAD + 8, PAD:PAD + S],
                        0.0)
                # ACT queue: keeps the SP queue free for plane prefetches so
                # the next rep's loads overlap this rep's fixup tail.
                nc.scalar.dma_start(y_d[:, p], opl[:])

            rep_ctx = tc.For_i(0, reps, 1) if reps > 1 else contextlib.nullcontext()
            with rep_ctx:
                for q in range(K):
                    load_plane(q)
                for p in range(XS):
                    if p + K < XIN:
                        load_plane(p + K)
                    if p >= 1:
                        stage1(p - 1)
                    if p >= 2:
                        stage2(p - 2)
                    conv(p)
                stage1(XS - 1, shifts=False)
                stage2(XS - 2)
                stage2(XS - 1, direct=True)

    nc.compile()
    return nc


def _host_inputs(x, w_gate):
    """Build the 8 per-core input maps."""
    import ml_dtypes

    e4 = ml_dtypes.float8_e4m3
    x = np.ascontiguousarray(x, dtype=np.float32)
    w_gate = np.ascontiguousarray(w_gate, dtype=np.float32)

    pairs, taps16 = _tap_split()
    n16 = len(taps16)

    # Wp[a, b][cin, kz*24+o] = w_gate[o, cin, a, b, kz] * WS
    wp = np.transpose(w_gate * WS, (2, 3, 1, 4, 0)).reshape(K, K, C, K * NCO)
    w16 = np.zeros((CIN, n16, 128), dtype=np.float16)
    for i, (a, b) in enumerate(taps16):
        w16[:C, i, :K * NCO] = wp[a, b].astype(np.float16)
    w8 = np.zeros((CIN, max(N_PAIRS, 1), 2, 128), dtype=e4)
    for j, (a, b0) in enumerate(pairs):
        for s in (0, 1):
            w8[:C, j, s, :K * NCO] = wp[a, b0 + s].astype(e4)

    def gate_of(m):
        if 32 <= m < 80:
            return (m - 32) // 3
        if 80 <= m < 120:
            return 16 + (m - 80) // 5
        return None

    # [:, 0, :] = combined selector; [:, 1+c, :] = kz-group c only (for the
    # last plane's direct-from-ss reduction).
    esum = np.zeros((CIN, 6, 128), dtype=np.float16)
    inv = np.float16(1.0 / WS)
    for kz in range(K):
        for m in range(C):
            o = gate_of(m)
            if o is not None:
                esum[kz * NCO + o, 0, m] = inv
                esum[kz * NCO + o, 1 + kz, m] = inv

    in_maps = []
    for i in range(N_CORES):
        b = i // NXS
        x0 = (i % NXS) * XS
        slab = np.zeros((CIN, XIN, SP, SP), dtype=np.float32)
        s0 = max(0, x0 - PAD)
        s1 = min(S, x0 + XS + PAD)
        d0 = s0 - (x0 - PAD)
        slab[:C, d0:d0 + (s1 - s0), PAD:PAD + S, PAD:PAD + S] = x[b, :, s0:s1]
        in_maps.append({
            "xs16": slab.astype(np.float16).reshape(CIN, XIN, FLAT),
            "xs8": slab.astype(e4).reshape(CIN, XIN, FLAT),
            "w16": w16, "w8": w8, "esum": esum,
        })
    return in_maps


def kernel(x, w_gate):
    import time

    from concourse.bass_utils import run_bass_kernel_spmd

    if "nc" not in _CACHE:
        _CACHE["nc"] = _build()
    nc = _CACHE["nc"]

    in_maps = _host_inputs(x, w_gate)
    last_err = None
    for attempt in range(3):
        try:
            res = run_bass_kernel_spmd(nc, in_maps, core_ids=list(range(N_CORES)))
            break
        except Exception as e:  # transient NRT device wedges recover on retry
            last_err = e
            time.sleep(5.0)
    else:
        raise last_err
    kernel._last_results = res

    out = np.empty((B, C, S, S, S), dtype=np.float32)
    for i in range(N_CORES):
        b = i // NXS
        x0 = (i % NXS) * XS
        out[b, :, x0:x0 + XS] = res.results[i]["y"].astype(np.float32)
    return out


# revision 24
# speedup vs baseline: 1.1885x; 1.1885x over previous
"""Trainium2 Bass kernel for GatedActivation (gate-conv3d + sigmoid gating).

Reference computation:
  x: [2, 120, 48, 48, 48] f32   (channels = 32 scalar + 16*3 (l=1) + 8*5 (l=2))
  w_gate: [24, 120, 5, 5, 5] f32
  g = sigmoid(conv3d(x, w_gate, same padding))         # [2, 24, 48, 48, 48]
  out[:, 0:32]    = relu(x[:, 0:32])
  out[:, 32+3m+d] = x[:, 32+3m+d] * g[:, m]            (m in 0..15)
  out[:, 80+5m+d] = x[:, 80+5m+d] * g[:, 16+m]         (m in 0..7)

Sharding: 8 cores = batch(2) x X-split(4 slabs of 12 planes).

Algorithm per core:
  - conv taps (a, b) = (kx, ky) with kz folded into the 120 stationary
    columns (kz*24+o).  The padded (y, z) plane (52*52=2704 flat cols) is
    streamed FLAT; the useful output region is flat cols [104, 2600),
    covered by 5 PSUM-bank chunks of 512/448 columns.  A tap's moving
    operand is the flat plane shifted by (b-2)*52 + chunk offset.
  - mixed precision: N_PAIRS tap-pairs (b0, b0+1) run as fp8e4m3 DoubleRow
    matmuls (moving [128, 2, N] carries both taps, 2 MACs/cell/cycle); the
    remaining taps run as fp16 matmuls.  Both accumulate into the same
    fp32 PSUM chunk banks.  Weights are pre-scaled by 512 so fp8 weights
    escape the e4m3 subnormal range; the esum selector divides it back.
    The fp8 share keeps conv quantization error inside the 2e-2 gate
    (err ~ 0.023 * sqrt(n8/25)).
  - fixup per PLANE (cuts DMA instruction count ~6x vs per-y-tile):
    5 ACT copies move the chunk banks to contiguous fp16 ss[128, 2496];
    5 gpsimd SBUF->SBUF DMAs extract per-kz-group z-aligned ss2[128,48,48]
    (group kz at partitions 24kz..24kz+24, source col 52*yy + z + kz);
    one esum matmul per y-tile sums the kz partials across partitions and
    broadcasts gates onto their 3/5 channels; sigmoid (ACT), gating
    multiply (DVE, fp16 out), relu overwrite (ACT), one output DMA per
    plane.  Output fp16, upcast to f32 on host.
"""

import sys

if "/opt/trn_rl_repo" not in sys.path:
    sys.path.insert(0, "/opt/trn_rl_repo")

import numpy as np

B = 2
C = 120
S = 48
K = 5
PAD = 2
NXS = 4          # x-axis shards
XS = S // NXS    # 12 output x-planes per core
XIN = XS + 2 * PAD   # 16 input planes per core
SP = S + 2 * PAD     # 52 padded y/z extent
FLAT = SP * SP       # 2704
NCO = 24
NSC = 32
CIN = 128
N_CORES = 8

# fp8 DoubleRow tap pairs (a, b0) = taps (a,b0)+(a,b0+1); 8 pairs = 16 of the
# 25 taps in fp8, the rest fp16.  This specific assignment (center x-plane
# a=2 kept fp16) measured the lowest max error of the sampled splits.
PAIRS_CFG = [(0, 1), (0, 3), (1, 1), (1, 3), (3, 0), (3, 2), (4, 0), (4, 3)]
N_PAIRS = len(PAIRS_CFG)
ALTERNATE = False    # grouped fp16-then-fp8 per chunk (mode switches cost HW time)
WS = 512.0           # weight prescale (e4m3 subnormal avoidance)

CH_N = [512, 512, 512, 512, 448]                # chunk widths
CH_OFF = [104, 616, 1128, 1640, 2152]           # absolute flat offsets

_CACHE = {}


def _tap_split():
    pairs = list(PAIRS_CFG)
    in_pair = {(a, b0 + i) for a, b0 in pairs for i in (0, 1)}
    assert len(in_pair) == 2 * len(pairs)
    taps16 = [(a, b) for a in range(K) for b in range(K) if (a, b) not in in_pair]
    return pairs, taps16


def _build(reps=1):
    import contextlib

    import concourse.tile as tile
    from concourse import bacc, mybir

    f32 = mybir.dt.float32
    f16 = mybir.dt.float16
    f8 = mybir.dt.float8e4

    pairs, taps16 = _tap_split()
    n16 = len(taps16)

    nc = bacc.Bacc("TRN2", target_bir_lowering=False, debug=False,
                   num_devices=N_CORES)
    xs16_d = nc.dram_tensor("xs16", [CIN, XIN, FLAT], f16, kind="ExternalInput").ap()
    xs8_d = nc.dram_tensor("xs8", [CIN, XIN, FLAT], f8, kind="ExternalInput").ap()
    w16_d = nc.dram_tensor("w16", [CIN, n16, 128], f16, kind="ExternalInput").ap()
    w8_d = nc.dram_tensor("w8", [CIN, max(N_PAIRS, 1), 2, 128], f8,
                          kind="ExternalInput").ap()
    esum_d = nc.dram_tensor("esum", [CIN, 6, 128], f16, kind="ExternalInput").ap()
    y_d = nc.dram_tensor("y", [C, XS, S, S], f16, kind="ExternalOutput").ap()

    with tile.TileContext(nc) as tc:
        with tc.tile_pool(name="wpool", bufs=1) as wpool, \
             tc.tile_pool(name="p16", bufs=7) as p16_pool, \
             tc.tile_pool(name="p8", bufs=7) as p8_pool, \
             tc.tile_pool(name="convps", bufs=5, space="PSUM") as conv_pool, \
             tc.tile_pool(name="sspool", bufs=2) as ss_pool, \
             tc.tile_pool(name="ss2pool", bufs=2) as ss2_pool, \
             tc.tile_pool(name="gpreps", bufs=3, space="PSUM") as gpre_pool, \
             tc.tile_pool(name="gsig", bufs=3) as gsig_pool, \
             tc.tile_pool(name="outpl", bufs=2) as out_pool:

            # weights go on the ACT queue so the SP queue starts streaming
            # plane loads immediately at kernel head.
            w16_t = wpool.tile([CIN, n16, 128], f16)
            nc.scalar.dma_start(w16_t[:], w16_d[:])
            w8_t = wpool.tile([CIN, max(N_PAIRS, 1), 2, 128], f8)
            nc.scalar.dma_start(w8_t[:], w8_d[:])
            esum_t = wpool.tile([CIN, 6, 128], f16)
            nc.scalar.dma_start(esum_t[:], esum_d[:])

            p16s = {}
            p8s = {}
            banks_of = {}
            ss2_of = {}

            def load_plane(q):
                t16 = p16_pool.tile([CIN, FLAT], f16, tag="p16", name=f"p16_{q}")
                nc.sync.dma_start(t16[:], xs16_d[:, q])
                t8 = p8_pool.tile([CIN, 2, FLAT], f8, tag="p8", name=f"p8_{q}")
                nc.scalar.dma_start(t8[:, 0, :], xs8_d[:, q])
                # slot 1 = same plane shifted one y-row (the DoubleRow pair
                # partner tap b+1); the tail is never read.
                nc.scalar.dma_start(t8[:, 1, 0:FLAT - SP], xs8_d[:, q, SP:FLAT])
                p16s[q] = t16
                p8s[q] = t8

            def conv(p):
                banks = [conv_pool.tile([CIN, n], f32, tag="cps",
                                        name=f"cps{p}_{c}")
                         for c, n in enumerate(CH_N)]
                banks_of[p] = banks
                # chunk-outer so each bank completes early (drain overlaps the
                # rest of the plane); fp16/fp8 taps ALTERNATE so every
                # DoubleRow LDWEIGHTS (256 cols, no FWL) hides under the
                # previous fp16 matmul's longer stream.
                ops = []
                if ALTERNATE:
                    for i in range(max(n16, len(pairs))):
                        if i < n16:
                            ops.append(("16", i, taps16[i]))
                        if i < len(pairs):
                            ops.append(("8", i, pairs[i]))
                else:
                    ops = [("16", i, t) for i, t in enumerate(taps16)] + \
                          [("8", i, t) for i, t in enumerate(pairs)]
                nops = len(ops)
                for c, n in enumerate(CH_N):
                    for idx, (kind, i, tap) in enumerate(ops):
                        first = idx == 0
                        last = idx == nops - 1
                        if kind == "16":
                            a, b = tap
                            off = CH_OFF[c] + (b - PAD) * SP
                            nc.tensor.matmul(
                                banks[c][:], w16_t[:, i],
                                p16s[p + a][:, off:off + n],
                                start=first, stop=last)
                        else:
                            a, b0 = tap
                            off = CH_OFF[c] + (b0 - PAD) * SP
                            nc.tensor.matmul(
                                banks[c][:], w8_t[:, i],
                                p8s[p + a][:, :, off:off + n],
                                start=first, stop=last,
                                perf_mode=mybir.MatmulPerfMode.DoubleRow,
                                skip_group_check=True)

            ss_of = {}

            def stage1(p, shifts=True):
                banks = banks_of.pop(p)
                # 4 spare cols so the kz-shifted view below stays in-bounds
                ss = ss_pool.tile([CIN, 2500], f16, tag="ss", name=f"ss{p}")
                for c, n in enumerate(CH_N):
                    o = CH_OFF[c] - CH_OFF[0]
                    nc.scalar.copy(ss[:, o:o + n], banks[c][:])
                if not shifts:
                    ss_of[p] = ss
                    return
                # group kz keeps 52-wide rows: ss2[g, yy, z<48] = conv partial
                # for out (yy, z); src is one CONTIGUOUS 2496-col run per
                # partition (SWDGE descriptor count stays tiny).
                ss2 = ss2_pool.tile([CIN, S * SP], f16, tag="ss2", name=f"ss2{p}")
                for c in range(K):
                    np0 = NCO * c
                    np1 = NCO * (c + 1) if c < K - 1 else CIN
                    nc.gpsimd.dma_start(ss2[np0:np1, :],
                                        ss[np0:np1, c:c + S * SP])
                ss2_of[p] = ss2

            def stage2(p, direct=False):
                opl = out_pool.tile([C, S, S], f16, tag="opl", name=f"opl{p}")
                x16v = p16s[p + PAD][:].rearrange("p (y z) -> p y z", y=SP)
                if direct:
                    ss = ss_of.pop(p)
                else:
                    ss2 = ss2_of.pop(p)
                    ss2v = ss2[:].rearrange("p (y z) -> p y z", z=SP)
                for k in range(S // 8):
                    gpre = gpre_pool.tile([CIN, 8 * S], f32, tag="gpre",
                                          name=f"gpre{p}_{k}")
                    if direct:
                        # last plane: skip the shift DMAs; 5 accumulating
                        # matmuls with per-kz selector stationaries read the
                        # shifted windows straight out of ss.
                        for c in range(K):
                            mv = ss[:, 8 * SP * k + c:8 * SP * k + c + 8 * SP]
                            mv = mv.rearrange("p (y z) -> p y z", z=SP)[:, :, 0:S]
                            nc.tensor.matmul(
                                gpre[:], esum_t[:, 1 + c], mv,
                                start=(c == 0), stop=(c == K - 1))
                    else:
                        nc.tensor.matmul(
                            gpre[:], esum_t[:, 0], ss2v[:, k * 8:(k + 1) * 8, 0:S],
                            start=True, stop=True)
                    gsig = gsig_pool.tile([C, 8, S], f16, tag="gsig",
                                          name=f"gsig{p}_{k}")
                    nc.scalar.activation(
                        gsig[:], gpre[0:C, :].rearrange("p (y z) -> p y z", y=8),
                        mybir.ActivationFunctionType.Sigmoid)
                    nc.vector.tensor_mul(
                        opl[:, k * 8:k * 8 + 8, :],
                        x16v[0:C, k * 8 + PAD:k * 8 + PAD + 8, PAD:PAD + S],
                        gsig[:])
                    nc.vector.tensor_scalar_max(
                        opl[0:NSC, k * 8:k * 8 + 8, :],
                        x16v[0:NSC, k * 8 + PAD:k * 8 + PAD + 8, PAD:PAD + S],
                        0.0)
                # ACT queue: keeps the SP queue free for plane prefetches so
                # the next rep's loads overlap this rep's fixup tail.
                nc.scalar.dma_start(y_d[:, p], opl[:])

            rep_ctx = tc.For_i(0, reps, 1) if reps > 1 else contextlib.nullcontext()
            with rep_ctx:
                for q in range(K):
                    load_plane(q)
                for p in range(XS):
                    if p + K < XIN:
                        load_plane(p + K)
                    if p >= 1:
                        stage1(p - 1)
                    if p >= 2:
                        stage2(p - 2)
                    conv(p)
                stage1(XS - 1, shifts=False)
                stage2(XS - 2)
                stage2(XS - 1, direct=True)

    nc.compile()
    return nc


def _host_inputs(x, w_gate):
    """Build the 8 per-core input maps."""
    import ml_dtypes

    e4 = ml_dtypes.float8_e4m3
    x = np.ascontiguousarray(x, dtype=np.float32)
    w_gate = np.ascontiguousarray(w_gate, dtype=np.float32)

    pairs, taps16 = _tap_split()
    n16 = len(taps16)

    # Wp[a, b][cin, kz*24+o] = w_gate[o, cin, a, b, kz] * WS
    wp = np.transpose(w_gate * WS, (2, 3, 1, 4, 0)).reshape(K, K, C, K * NCO)
    w16 = np.zeros((CIN, n16, 128), dtype=np.float16)
    for i, (a, b) in enumerate(taps16):
        w16[:C, i, :K * NCO] = wp[a, b].astype(np.float16)
    w8 = np.zeros((CIN, max(N_PAIRS, 1), 2, 128), dtype=e4)
    for j, (a, b0) in enumerate(pairs):
        for s in (0, 1):
            w8[:C, j, s, :K * NCO] = wp[a, b0 + s].astype(e4)

    def gate_of(m):
        if 32 <= m < 80:
            return (m - 32) // 3
        if 80 <= m < 120:
            return 16 + (m - 80) // 5
        return None

    # [:, 0, :] = combined selector; [:, 1+c, :] = kz-group c only (for the
    # last plane's direct-from-ss reduction).
    esum = np.zeros((CIN, 6, 128), dtype=np.float16)
    inv = np.float16(1.0 / WS)
    for kz in range(K):
        for m in range(C):
            o = gate_of(m)
            if o is not None:
                esum[kz * NCO + o, 0, m] = inv
                esum[kz * NCO + o, 1 + kz, m] = inv

    in_maps = []
    for i in range(N_CORES):
        b = i // NXS
        x0 = (i % NXS) * XS
        slab = np.zeros((CIN, XIN, SP, SP), dtype=np.float32)
        s0 = max(0, x0 - PAD)
        s1 = min(S, x0 + XS + PAD)
        d0 = s0 - (x0 - PAD)
        slab[:C, d0:d0 + (s1 - s0), PAD:PAD + S, PAD:PAD + S] = x[b, :, s0:s1]
        in_maps.append({
            "xs16": slab.astype(np.float16).reshape(CIN, XIN, FLAT),
            "xs8": slab.astype(e4).reshape(CIN, XIN, FLAT),
            "w16": w16, "w8": w8, "esum": esum,
        })
    return in_maps


def kernel(x, w_gate):
    import time

    from concourse.bass_utils import run_bass_kernel_spmd

    if "nc" not in _CACHE:
        _CACHE["nc"] = _build()
    nc = _CACHE["nc"]

    in_maps = _host_inputs(x, w_gate)
    last_err = None
    for attempt in range(3):
        try:
            res = run_bass_kernel_spmd(nc, in_maps, core_ids=list(range(N_CORES)))
            break
        except Exception as e:  # transient NRT device wedges recover on retry
            last_err = e
            time.sleep(5.0)
    else:
        raise last_err
    kernel._last_results = res

    out = np.empty((B, C, S, S, S), dtype=np.float32)
    for i in range(N_CORES):
        b = i // NXS
        x0 = (i % NXS) * XS
        out[b, :, x0:x0 + XS] = res.results[i]["y"].astype(np.float32)
    return out
